# revision 1
# baseline (speedup 1.0000x reference)
"""Trainium2 Bass kernel for nn_Model4 (5-step GCN) — design S.

Differences from the v1 kernel:
  - Edge chunks are dest-boundary-aligned windows of <=CE edges per band,
    with boundaries shared across all cores (one SPMD program), so gather
    padding is ~0 (v1 padded every bucket to the global max).
  - Segment sums via RESET-SCAN (tensor_tensor_scan: state = m*state + g,
    m=0 at each dest's first edge); per-dest sums extracted at each dest's
    last-edge slot. Kills v1's memset + boundary-shift + subtract.
  - ReduceScatter split into NWIN dest-window collectives, each issued as
    soon as every band has filled the window -> communication overlaps the
    remaining chunks' gathers; update phase runs per window, also under
    the gathers' shadow. Feature tables ping-pong so updates never wait
    on this step's gather reads.
  - wgb.T @ (dinv*b) is constant across steps: computed once in init into
    DRAM (htb) and added per update chunk; per-step wgb matmuls gone.
"""
import os

import numpy as np

import concourse.bacc as bacc
import concourse.mybir as mybir
import concourse.tile as tile
from concourse.bass_utils import run_bass_kernel_spmd

N = 100000
NCORES = 8
NPC = N // NCORES              # 12500 nodes per core
PADN = 12544                   # padded per-core node count (98*128)
ZCOL = PADN - 1                # all-zero table column (pads land here)
NSTEP = int(os.environ.get("K2_NSTEP", "5"))
CE = int(os.environ.get("K2_CE", "3200"))   # edge slots per chunk per band
CAP = CE - 16                  # chunk edge capacity (>=1 pad slot)
TAB2 = bool(int(os.environ.get("K2_TAB2", "1")))
UW = 512                       # update sub-chunk width
MW = 512                       # matmul/psum sub-width
CW = 512                       # init chunk width
F32 = mybir.dt.float32
BF16 = mybir.dt.bfloat16
I16 = mybir.dt.int16
AX = mybir.AluOpType
ACTF = mybir.ActivationFunctionType

_cache = {}


def _wrap16(vals, ni):
    flat = np.zeros(ni, dtype=np.int64)
    flat[:len(vals)] = vals
    return flat.reshape(ni // 16, 16).T.astype(np.int16)


def _preprocess(edges):
    row = np.ascontiguousarray(edges[0]).astype(np.int64)
    col = np.ascontiguousarray(edges[1]).astype(np.int64)
    deg = (np.bincount(col, minlength=N) + 1).astype(np.float32)
    dinv = deg ** np.float32(-0.5)

    core = row // NPC
    band = col // NPC
    ls = (row - core * NPC).astype(np.int32)
    ld = (col - band * NPC).astype(np.int32)

    # per (core, band): dest-sorted edge lists + per-dest cumulative counts
    data = [[None] * NCORES for _ in range(NCORES)]
    for i in range(NCORES):
        for k in range(NCORES):
            sel = (core == i) & (band == k)
            ldk = ld[sel]
            lsk = ls[sel][np.argsort(ldk, kind="stable")]
            ldk = np.sort(ldk, kind="stable")
            degk = np.bincount(ldk, minlength=PADN)
            cum = np.concatenate([[0], np.cumsum(degk)]).astype(np.int64)
            data[i][k] = (lsk, degk, cum)

    # shared chunk boundaries per band: D[k][c], valid for every core
    D = []
    NCHK = 0
    for k in range(NCORES):
        bd = [0]
        while bd[-1] < PADN:
            d_hi = PADN
            for i in range(NCORES):
                cum = data[i][k][2]
                h = int(np.searchsorted(cum, cum[bd[-1]] + CAP,
                                        side="right")) - 1
                d_hi = min(d_hi, h)
            assert d_hi > bd[-1]
            bd.append(min(d_hi, PADN))
        D.append(bd)
        NCHK = max(NCHK, len(bd) - 1)
    for k in range(NCORES):
        while len(D[k]) < NCHK + 1:
            D[k].append(PADN)

    # shared per-chunk shapes
    ni_list, bd_list = [], []
    for c in range(NCHK):
        mx_slots = max(
            int(data[i][k][2][D[k][c + 1]] - data[i][k][2][D[k][c]])
            for i in range(NCORES) for k in range(NCORES))
        mx_dests = max(D[k][c + 1] - D[k][c] for k in range(NCORES))
        ni_list.append(((mx_slots + 1 + 15) // 16) * 16)
        # mult of 32 so per-chunk int16 idx slices stay 4B-aligned for
        # the Q7 idx-load (it reads the wrapped list as 32-bit words)
        bd_list.append(max(((mx_dests + 31) // 32) * 32, 32))

    # rs windows: close after chunk counts ~ [1/2, 3/4, 7/8, 1]*NCHK
    rs_after = sorted(set(
        max(1, (NCHK * f + den - 1) // den)
        for f, den in ((1, 2), (3, 4), (7, 8), (15, 16), (1, 1))))
    if rs_after[-1] != NCHK:
        rs_after[-1] = NCHK
    win_edges = [0]
    for c in rs_after[:-1]:
        m = min(D[k][min(c, NCHK)] for k in range(NCORES))
        if m > win_edges[-1]:
            win_edges.append(m)
    win_edges.append(PADN)
    # re-derive close points: window w closes after chunk c iff all bands
    # have D[k][c] >= win_edges[w+1]
    rs_close = {}
    for w in range(len(win_edges) - 1):
        need = win_edges[w + 1]
        c_ok = next(c for c in range(1, NCHK + 1)
                    if all(D[k][c] >= need for k in range(NCORES)))
        rs_close.setdefault(c_ok, []).append(w)

    # rs write slices per (band, chunk): (b_lo, b_hi, win, col_lo) shared
    rs_slices = [[[] for _ in range(NCHK)] for _ in range(NCORES)]
    for k in range(NCORES):
        for c in range(NCHK):
            d0, d1 = D[k][c], D[k][c + 1]
            for w in range(len(win_edges) - 1):
                wlo, whi = win_edges[w], win_edges[w + 1]
                s_lo, s_hi = max(d0, wlo), min(d1, whi)
                if s_lo < s_hi:
                    rs_slices[k][c].append((s_lo - d0, s_hi - d0, w, s_lo - wlo))

    # per-core device tensors
    bdtot = sum(bd_list)
    cores = []
    bf16 = mybir.dt.np(BF16)
    for i in range(NCORES):
        eidx = np.zeros((128, NCHK, CE // 16), dtype=np.int16)
        bidx = np.zeros((128, bdtot // 16), dtype=np.int16)
        mres = np.ones((NCHK, 128, CE), dtype=np.float32)
        for k in range(NCORES):
            lsk, degk, cum = data[i][k]
            for c in range(NCHK):
                d0, d1 = D[k][c], D[k][c + 1]
                e0, e1 = int(cum[d0]), int(cum[d1])
                n = e1 - e0
                ni = ni_list[c]
                g = np.full(ni, ZCOL, dtype=np.int64)
                g[:n] = lsk[e0:e1]
                eidx[16 * k:16 * (k + 1), c, :ni // 16] = _wrap16(g, ni)
                m = np.ones(CE, dtype=np.float32)
                dd = degk[d0:d1]
                if len(dd):
                    starts = np.concatenate([[0], np.cumsum(dd)])[:-1][dd > 0]
                    m[starts] = 0.0
                m[n:] = 0.0
                mres[c, 16 * k:16 * (k + 1), :] = m[None, :]
                bd = bd_list[c]
                off = sum(bd_list[:c])
                if d1 > d0:
                    ends = np.cumsum(dd) - 1
                    ends = np.where(dd > 0, ends, n)
                    e_arr = np.full(bd, n, dtype=np.int64)
                    e_arr[:d1 - d0] = ends
                else:
                    e_arr = np.full(bd, n, dtype=np.int64)
                bidx[16 * k:16 * (k + 1), off // 16:(off + bd) // 16] = \
                    _wrap16(e_arr, bd)
        cores.append(dict(eidx=eidx, bidx=bidx, mres=mres.astype(bf16)))

    meta = dict(NCHK=NCHK, ni_list=ni_list, bd_list=bd_list,
                win_edges=win_edges, rs_close=rs_close, rs_slices=rs_slices)
    return cores, dinv, meta


def _build(meta):
    NCHK = meta["NCHK"]
    ni_list = meta["ni_list"]
    bd_list = meta["bd_list"]
    win_edges = meta["win_edges"]
    rs_close = meta["rs_close"]
    rs_slices = meta["rs_slices"]
    NWIN = len(win_edges) - 1
    bdtot = sum(bd_list)
    bd_max = max(bd_list)

    nc = bacc.Bacc("TRN2", target_bir_lowering=False, debug=False,
                   num_devices=NCORES)

    xin_d = nc.dram_tensor("xin", [19, PADN], F32, kind="ExternalInput")
    dinv_d = nc.dram_tensor("dinv", [19, PADN], F32, kind="ExternalInput")
    eidx_d = nc.dram_tensor("eidx", [128, NCHK, CE // 16], I16,
                            kind="ExternalInput")
    bidx_d = nc.dram_tensor("bidx", [128, bdtot // 16], I16,
                            kind="ExternalInput")
    mres_d = nc.dram_tensor("mres", [NCHK, 128, CE], BF16,
                            kind="ExternalInput")
    w1_d = nc.dram_tensor("w1", [15, 15], F32, kind="ExternalInput")
    wga_d = nc.dram_tensor("wga", [15, 15], F32, kind="ExternalInput")
    wgb_d = nc.dram_tensor("wgb", [19, 15], F32, kind="ExternalInput")
    w4_d = nc.dram_tensor("w4", [19, 19], F32, kind="ExternalInput")
    w3a_d = nc.dram_tensor("w3a", [15, 1], F32, kind="ExternalInput")
    w3b_d = nc.dram_tensor("w3b", [19, 1], F32, kind="ExternalInput")
    b1_d = nc.dram_tensor("b1", [15, 1], F32, kind="ExternalInput")
    bg_d = nc.dram_tensor("bg", [15, 1], F32, kind="ExternalInput")
    b4_d = nc.dram_tensor("b4", [19, 1], F32, kind="ExternalInput")

    part_d = nc.dram_tensor("part", [1, 1], F32, kind="ExternalOutput")
    if os.environ.get("K2_DEBUG"):
        dbg_d = nc.dram_tensor("dbg", [16, PADN], F32, kind="ExternalOutput")

    htb_d = nc.dram_tensor("htb", [16, PADN], F32)
    rs_in = [nc.dram_tensor(f"rs_in{w}", [128, win_edges[w + 1] - win_edges[w]],
                            F32) for w in range(NWIN)]
    rs_out = [nc.dram_tensor(f"rs_out{w}", [16, win_edges[w + 1] - win_edges[w]],
                             F32) for w in range(NWIN)]

    with tile.TileContext(nc) as tc:
        with (
            tc.tile_pool(name="static", bufs=1) as st,
            tc.tile_pool(name="gbuf", bufs=2) as gbuf,
            tc.tile_pool(name="pbuf", bufs=1) as pbuf,
            tc.tile_pool(name="mbuf", bufs=2) as mbuf,
            tc.tile_pool(name="bbuf", bufs=2) as bbuf,
            tc.tile_pool(name="nbuf", bufs=2) as nbuf,
            tc.tile_pool(name="ps", bufs=2, space="PSUM") as ps,
            tc.tile_pool(name="ps2", bufs=3, space="PSUM") as ps2,
        ):
            NTAB = 2 if TAB2 else 1
            TABS = [st.tile([128, PADN], F32, name=f"TAB{t}")
                    for t in range(NTAB)]
            EIDX = st.tile([128, NCHK, CE // 16], I16)
            BIDX = st.tile([128, bdtot // 16], I16)
            w1 = st.tile([15, 15], F32)
            wga = st.tile([15, 15], F32)
            wgb = st.tile([19, 15], F32)
            w4 = st.tile([19, 19], F32)
            w3a = st.tile([15, 1], F32)
            w3b = st.tile([19, 1], F32)
            b1 = st.tile([15, 1], F32)
            bg = st.tile([15, 1], F32)
            b4 = st.tile([19, 1], F32)
            acc = st.tile([1, 1], F32)

            for t in TABS:
                nc.vector.memset(t[:], 0.0)
            nc.vector.memset(acc[:], 0.0)
            nc.sync.dma_start(out=EIDX[:], in_=eidx_d[:])
            nc.sync.dma_start(out=BIDX[:], in_=bidx_d[:])
            for t, d in ((w1, w1_d), (wga, wga_d), (wgb, wgb_d), (w4, w4_d),
                         (w3a, w3a_d), (w3b, w3b_d), (b1, b1_d), (bg, bg_d),
                         (b4, b4_d)):
                nc.sync.dma_start(out=t[:], in_=d[:])

            def readout(psr_tile, cols):
                red = nbuf.tile([1, 1], F32, tag="red")
                nc.vector.tensor_reduce(
                    out=red[:], in_=psr_tile[0:1, 0:cols],
                    axis=mybir.AxisListType.X, op=AX.add)
                nc.vector.tensor_add(acc[:], acc[:], red[:])

            # ---- init: b readout, htb, a0, first table ----
            TAB = TABS[0]
            for j0 in range(0, PADN, CW):
                j1 = min(j0 + CW, PADN)
                wd = j1 - j0
                sl = slice(j0, j1)
                real = max(min(NPC - j0, wd), 0)
                xin_t = nbuf.tile([19, CW], F32, tag="t19a")
                dv_t = nbuf.tile([19, CW], F32, tag="t16a")
                nc.sync.dma_start(out=xin_t[:, 0:wd], in_=xin_d[:, sl])
                nc.scalar.dma_start(out=dv_t[:, 0:wd], in_=dinv_d[:, sl])
                psb = ps.tile([19, CW], F32, tag="ps19")
                nc.tensor.matmul(out=psb[:, 0:wd], lhsT=w4[:],
                                 rhs=xin_t[:, 0:wd], start=True, stop=True)
                bt_t = nbuf.tile([19, CW], F32, tag="t19b")
                nc.scalar.activation(out=bt_t[:, 0:wd], in_=psb[:, 0:wd],
                                     func=ACTF.Identity, bias=b4[:], scale=1.0)
                if real > 0:
                    psr = ps.tile([1, CW], F32, tag="ps1")
                    nc.tensor.matmul(out=psr[:, 0:wd], lhsT=w3b[:],
                                     rhs=bt_t[:, 0:wd], start=True, stop=True)
                    readout(psr, real)
                btt = nbuf.tile([19, CW], F32, tag="t19c")
                nc.vector.tensor_mul(btt[:, 0:wd], bt_t[:, 0:wd],
                                     dv_t[:, 0:wd])
                psh2 = ps2.tile([15, CW], F32, tag="ps15")
                nc.tensor.matmul(out=psh2[:, 0:wd], lhsT=wgb[:],
                                 rhs=btt[:, 0:wd], start=True, stop=True)
                htb_t = nbuf.tile([16, CW], F32, tag="t16b")
                nc.vector.memset(htb_t[:], 0.0)
                nc.scalar.copy(out=htb_t[0:15, 0:wd], in_=psh2[:, 0:wd])
                nc.sync.dma_start(out=htb_d[:, sl], in_=htb_t[:, 0:wd])
                psa = ps2.tile([15, CW], F32, tag="ps15")
                nc.tensor.matmul(out=psa[:, 0:wd], lhsT=w1[:],
                                 rhs=xin_t[0:15, 0:wd], start=True, stop=True)
                a_t = nbuf.tile([15, CW], F32, tag="t15a")
                nc.scalar.activation(out=a_t[:, 0:wd], in_=psa[:, 0:wd],
                                     func=ACTF.Relu, bias=b1[:], scale=1.0)
                at_t = nbuf.tile([15, CW], F32, tag="t15b")
                nc.vector.tensor_mul(at_t[:, 0:wd], a_t[:, 0:wd],
                                     dv_t[0:15, 0:wd])
                psh = ps2.tile([15, CW], F32, tag="ps15")
                nc.tensor.matmul(out=psh[:, 0:wd], lhsT=wga[:],
                                 rhs=at_t[:, 0:wd], start=True, stop=True)
                nc.vector.tensor_add(TAB[0:15, sl], psh[:, 0:wd],
                                     htb_t[0:15, 0:wd])
            for k in range(1, 8):
                nc.sync.dma_start(out=TAB[16 * k:16 * (k + 1), :],
                                  in_=TAB[0:16, :])

            # ---- steps ----
            for s in range(NSTEP):
                TAB = TABS[s % NTAB]
                TABN = TABS[(s + 1) % NTAB]
                last = s == NSTEP - 1
                gs, ms = {}, {}

                def issue_gather(c, gs=gs, ms=ms, TAB=TAB):
                    ni = ni_list[c]
                    G = gbuf.tile([128, CE], F32, tag="G")
                    nc.gpsimd.ap_gather(
                        out_ap=G[:, 0:ni], in_ap=TAB[:],
                        idxs_ap=EIDX[:, c, 0:ni // 16],
                        channels=128, num_elems=PADN, d=1, num_idxs=ni)
                    M = mbuf.tile([128, CE], BF16, tag="M")
                    nc.scalar.dma_start(out=M[:], in_=mres_d[c])
                    gs[c] = G
                    ms[c] = M

                def do_update_window(w, TAB=TAB, TABN=TABN, last=last):
                    wlo, whi = win_edges[w], win_edges[w + 1]
                    for j0 in range(wlo, whi, UW):
                        j1 = min(j0 + UW, whi)
                        wdt = j1 - j0
                        real = max(min(NPC - j0, wdt), 0)
                        st_t = nbuf.tile([16, UW], F32, tag="t16c")
                        dv_t = nbuf.tile([16, UW], F32, tag="t16a")
                        nc.sync.dma_start(
                            out=st_t[:, 0:wdt],
                            in_=rs_out[w][:, j0 - wlo:j1 - wlo])
                        nc.scalar.dma_start(out=dv_t[:, 0:wdt],
                                            in_=dinv_d[0:16, j0:j1])
                        t1 = nbuf.tile([15, UW], F32, tag="t15a")
                        nc.vector.tensor_add(t1[:, 0:wdt], st_t[0:15, 0:wdt],
                                             TAB[0:15, j0:j1])
                        nc.vector.tensor_mul(t1[:, 0:wdt], t1[:, 0:wdt],
                                             dv_t[0:15, 0:wdt])
                        a_t = nbuf.tile([15, UW], F32, tag="t15b")
                        nc.scalar.activation(out=a_t[:, 0:wdt],
                                             in_=t1[:, 0:wdt],
                                             func=ACTF.Relu, bias=bg[:],
                                             scale=1.0)
                        if last:
                            if os.environ.get("K2_DEBUG"):
                                nc.sync.dma_start(out=dbg_d[0:15, j0:j1],
                                                  in_=a_t[:, 0:wdt])
                            for q0 in range(0, wdt, MW):
                                q1 = min(q0 + MW, wdt)
                                rq = max(min(real - q0, q1 - q0), 0)
                                if rq > 0:
                                    psr = ps.tile([1, MW], F32, tag="ps1")
                                    nc.tensor.matmul(
                                        out=psr[:, 0:q1 - q0], lhsT=w3a[:],
                                        rhs=a_t[:, q0:q1],
                                        start=True, stop=True)
                                    readout(psr, rq)
                        else:
                            nc.vector.tensor_mul(a_t[:, 0:wdt], a_t[:, 0:wdt],
                                                 dv_t[0:15, 0:wdt])
                            htb_t = nbuf.tile([16, UW], F32, tag="t16b")
                            nc.scalar.dma_start(out=htb_t[:, 0:wdt],
                                                in_=htb_d[:, j0:j1])
                            for q0 in range(0, wdt, MW):
                                q1 = min(q0 + MW, wdt)
                                psh = ps2.tile([15, MW], F32, tag="ps15")
                                nc.tensor.matmul(out=psh[:, 0:q1 - q0],
                                                 lhsT=wga[:],
                                                 rhs=a_t[:, q0:q1],
                                                 start=True, stop=True)
                                nc.vector.tensor_add(
                                    TABN[0:15, j0 + q0:j0 + q1],
                                    psh[:, 0:q1 - q0],
                                    htb_t[0:15, q0:q1])
                    if not last:
                        for k in range(1, 8):
                            nc.sync.dma_start(
                                out=TABN[16 * k:16 * (k + 1), wlo:whi],
                                in_=TABN[0:16, wlo:whi])

                issue_gather(0)
                boff = 0
                for c in range(NCHK):
                    ni = ni_list[c]
                    bd = bd_list[c]
                    if c + 1 < NCHK:
                        issue_gather(c + 1)
                    G = gs.pop(c)
                    M = ms.pop(c)
                    P = pbuf.tile([128, CE], F32, tag="P")
                    nc.vector.tensor_tensor_scan(
                        out=P[:, 0:ni], data0=M[:, 0:ni], data1=G[:, 0:ni],
                        initial=0.0, op0=AX.mult, op1=AX.add)
                    B = bbuf.tile([128, bd_max], F32, tag="B")
                    nc.gpsimd.ap_gather(
                        out_ap=B[:, 0:bd], in_ap=P[:, 0:ni],
                        idxs_ap=BIDX[:, boff // 16:(boff + bd) // 16],
                        channels=128, num_elems=ni, d=1, num_idxs=bd)
                    for k in range(NCORES):
                        for (b_lo, b_hi, w, col_lo) in rs_slices[k][c]:
                            nc.sync.dma_start(
                                out=rs_in[w][16 * k:16 * (k + 1),
                                             col_lo:col_lo + (b_hi - b_lo)],
                                in_=B[16 * k:16 * (k + 1), b_lo:b_hi])
                    boff += bd
                    for w in rs_close.get(c + 1, ()):
                        nc.gpsimd.collective_compute(
                            "ReduceScatter", AX.add,
                            replica_groups=[list(range(NCORES))],
                            ins=[rs_in[w][:]], outs=[rs_out[w][:]])
                for w in range(NWIN):
                    do_update_window(w)

            nc.sync.dma_start(out=part_d[:], in_=acc[:])
    nc.finalize()
    return nc


def kernel(**inputs):
    x1 = np.ascontiguousarray(np.asarray(inputs["x1"], dtype=np.float32))
    x2 = np.ascontiguousarray(np.asarray(inputs["x2"], dtype=np.float32))
    edges = np.asarray(inputs["edges"])
    W1 = np.asarray(inputs["W1"], dtype=np.float32)
    b1 = np.asarray(inputs["b1"], dtype=np.float32)
    Wg = np.asarray(inputs["Wg"], dtype=np.float32)
    bg = np.asarray(inputs["bg"], dtype=np.float32)
    W3 = np.asarray(inputs["W3"], dtype=np.float32)
    b3 = np.asarray(inputs["b3"], dtype=np.float32)
    W4 = np.asarray(inputs["W4"], dtype=np.float32)
    b4 = np.asarray(inputs["b4"], dtype=np.float32)

    cores, dinv, meta = _preprocess(edges)

    x2t = np.tile(x2, (20, 1))
    xin = np.concatenate([x1, x2t], axis=1)     # [N, 19]

    in_maps = []
    for i in range(NCORES):
        sl = slice(NPC * i, NPC * (i + 1))
        xinT = np.zeros((19, PADN), dtype=np.float32)
        xinT[:, :NPC] = xin[sl].T
        dvT = np.zeros((19, PADN), dtype=np.float32)
        dvT[:, :NPC] = np.broadcast_to(dinv[sl], (19, NPC))
        in_maps.append(dict(
            xin=xinT, dinv=dvT,
            eidx=cores[i]["eidx"], bidx=cores[i]["bidx"],
            mres=cores[i]["mres"],
            w1=np.ascontiguousarray(W1.T),
            wga=np.ascontiguousarray(Wg[:, :15].T),
            wgb=np.ascontiguousarray(Wg[:, 15:].T),
            w4=np.ascontiguousarray(W4.T),
            w3a=np.ascontiguousarray(W3[0, :15, None]),
            w3b=np.ascontiguousarray(W3[0, 15:, None]),
            b1=np.ascontiguousarray(b1[:, None]),
            bg=np.ascontiguousarray(bg[:, None]),
            b4=np.ascontiguousarray(b4[:, None]),
        ))

    key = (meta["NCHK"], tuple(meta["ni_list"]), tuple(meta["bd_list"]),
           tuple(meta["win_edges"]))
    if key not in _cache:
        _cache[key] = _build(meta)
    nc = _cache[key]

    res = run_bass_kernel_spmd(nc, in_maps, list(range(NCORES))).results
    total = sum(float(res[i]["part"][0, 0]) for i in range(NCORES))
    out = np.tanh((total + N * float(b3.reshape(-1)[0])) / N)
    return np.float32(out)

